# revision 4
# baseline (speedup 1.0000x reference)
"""Causal multi-head attention TRN2 kernel.

Problem: B=4, S=2048, E=1024, H=16 heads (D=64), fused QKV projection
(torch-Linear convention), causal softmax attention, output projection.

Sharding (8 NeuronCores): data-parallel over B (4) x tensor-parallel over
heads (2 groups of 8 heads, Megatron-style column/row split of Wa/Wo).
Each core computes a full [S, E] partial of the output projection for its
(batch, head-group); the host sums the two head-group partials per batch
and adds the output bias.

On-core design (all matmuls fp32r = full-rate PE, ~1e-4 rel):
 - QKV projection computed feature-major for Q,K (qk^T = Wqk @ x^T) so the
   attention contraction dims land on SBUF partitions with no transposes,
   and sequence-major for V (v = x @ Wv^T) so V is ready as the AV lhsT.
 - scores^T[k,q] per head-pair: two Kc=64 matmuls packed on PE row groups
   (heads 2t / 2t+1 live on partitions 0:64 / 64:128 of one SBUF tile).
 - softmax without max-subtraction (scores are O(1) bounded for this
   distribution): exp on ACT (PSUM->SBUF eviction for free), causal mask
   applied by GPSIMD affine_select on the diagonal blocks only, softmax
   denominator for free as a 65th ones-column in the AV lhsT.
 - AV accumulates out_h^T[d,q] over key blocks in PSUM; normalization via
   DVE reciprocal + one Kc=1 broadcast matmul + DVE multiply.
 - Output projection consumes the head-major out^T tiles directly.
"""

import numpy as np

import concourse.bass as bass
import concourse.mybir as mybir
import concourse.tile as tile
from concourse import bacc
from concourse.bass_utils import run_bass_kernel_spmd

B, S, E, H = 4, 2048, 1024, 16
D = 64
HPC = 8  # heads per core
F32 = mybir.dt.float32
F32R = mybir.dt.float32r
EXP = mybir.ActivationFunctionType.Exp
ADD = mybir.AluOpType.add
MUL = mybir.AluOpType.mult
GE = mybir.AluOpType.is_ge

_NC = None


def _build():
    nc = bacc.Bacc()
    xT = nc.declare_dram_parameter("xT", [E, S], F32R, isOutput=False)
    wqk = nc.declare_dram_parameter("wqk", [E, E], F32R, isOutput=False)
    wv = nc.declare_dram_parameter("wv", [E, 512], F32R, isOutput=False)
    wo = nc.declare_dram_parameter("wo", [512, E], F32R, isOutput=False)
    bqk = nc.declare_dram_parameter("bqk", [128, 8], F32, isOutput=False)
    bv = nc.declare_dram_parameter("bv", [1, 512], F32R, isOutput=False)
    outT = nc.declare_dram_parameter("outT", [E, S], F32, isOutput=True)

    with tile.TileContext(nc) as tc:
        with tc.tile_pool(name="persist", bufs=1) as pers:
            # qk_sb[:, t<4, s]  : partitions 0:64 = Q^T of head 2t, 64:128 head 2t+1
            # qk_sb[:, 4+t, s]  : same for K^T
            qk_sb = pers.tile([128, 8, S], F32R)
            # v_sb[p, kb, h, d] : V[kb*128+p, h*64+d], col 64 = 1.0 (denominator)
            v_sb = pers.tile([128, 16, HPC, D + 1], F32R)
            bqk_sb = pers.tile([128, 8], F32)
            bvb_sb = pers.tile([128, 512], F32)
            ones_r = pers.tile([1, 128], F32R)
            onescol = pers.tile([128, 8], F32)
            ones32 = pers.tile([1, 128], F32)

            nc.sync.dma_start(bqk_sb[:], bqk[:])
            nc.vector.memset(ones32[:], 1.0)
            nc.vector.tensor_copy(ones_r[:], ones32[:])
            nc.vector.memset(onescol[:], 1.0)

            # ---- Phase 1: fused QKV projection ----
            with tc.tile_pool(name="ph1", bufs=1) as ph1, \
                 tc.tile_pool(name="xch", bufs=2) as xch, \
                 tc.tile_pool(name="pp", bufs=2, space="PSUM") as pp:
                wqk_sb = ph1.tile([128, 8, E], F32R)
                wv_sb = ph1.tile([128, 8, 512], F32R)
                bv_sb = ph1.tile([1, 512], F32R)
                nc.sync.dma_start(wqk_sb[:], wqk.rearrange("(eo p) f -> p eo f", p=128))
                nc.sync.dma_start(wv_sb[:], wv.rearrange("(eo p) f -> p eo f", p=128))
                nc.sync.dma_start(bv_sb[:], bv[:])
                # broadcast V bias across partitions with a Kc=1 matmul
                pb = pp.tile([128, 512], F32, tag="pp")
                nc.tensor.matmul(pb[:], ones_r[:], bv_sb[:], start=True, stop=True)
                nc.vector.tensor_copy(bvb_sb[:], pb[:])

                for s in range(4):
                    xc = xch.tile([128, 8, 512], F32R)
                    nc.sync.dma_start(
                        xc[:],
                        xT[:, s * 512:(s + 1) * 512].rearrange("(eo p) s -> p eo s", p=128),
                    )
                    for f in range(8):
                        pq = pp.tile([128, 512], F32, tag="pp")
                        for e in range(8):
                            nc.tensor.matmul(
                                pq[:],
                                wqk_sb[:, e, f * 128:(f + 1) * 128],
                                xc[:, e, :],
                                start=(e == 0), stop=(e == 7),
                            )
                        nc.vector.tensor_scalar_add(
                            qk_sb[:, f, s * 512:(s + 1) * 512], pq[:], bqk_sb[:, f:f + 1]
                        )
                    for smi in range(4):
                        sm = s * 4 + smi
                        pv = pp.tile([128, 512], F32, tag="pv")
                        for e in range(8):
                            nc.tensor.matmul(
                                pv[:],
                                xc[:, e, smi * 128:(smi + 1) * 128],
                                wv_sb[:, e, :],
                                start=(e == 0), stop=(e == 7),
                            )
                        nc.vector.tensor_tensor(
                            v_sb[:, sm, :, 0:D],
                            pv[:].rearrange("p (h d) -> p h d", h=HPC),
                            bvb_sb[:].rearrange("p (h d) -> p h d", h=HPC),
                            ADD,
                        )
                        nc.vector.tensor_copy(v_sb[:, sm, :, D:D + 1], onescol[:, :, None])

            # ---- Phase 2: attention + output projection ----
            with tc.tile_pool(name="ph2", bufs=1) as ph2, \
                 tc.tile_pool(name="aop", bufs=2) as aop, \
                 tc.tile_pool(name="expp", bufs=6) as expp, \
                 tc.tile_pool(name="rcp", bufs=4) as rcp, \
                 tc.tile_pool(name="oevp", bufs=4) as oevp, \
                 tc.tile_pool(name="scp", bufs=2, space="PSUM") as scp, \
                 tc.tile_pool(name="avp", bufs=4, space="PSUM") as avp:
                wo_sb = ph2.tile([128, 4, E], F32R)
                nc.sync.dma_start(wo_sb[:], wo.rearrange("(t p) f -> p t f", p=128))

                for qt in range(4):
                    ao = aop.tile([128, 4, 512], F32R)
                    for hp in range(4):
                        kmax = 4 * qt + 4
                        avA = avp.tile([65, 512], F32, tag="av")
                        avB = avp.tile([65, 512], F32, tag="av")
                        for kb in range(kmax):
                            sc = scp.tile([128, 1024], F32, tag="sc")
                            nc.tensor.matmul(
                                sc[:, 0:512],
                                qk_sb[0:64, 4 + hp, kb * 128:(kb + 1) * 128],
                                qk_sb[0:64, hp, qt * 512:(qt + 1) * 512],
                                start=True, stop=True,
                            )
                            nc.tensor.matmul(
                                sc[:, 512:1024],
                                qk_sb[64:128, 4 + hp, kb * 128:(kb + 1) * 128],
                                qk_sb[64:128, hp, qt * 512:(qt + 1) * 512],
                                start=True, stop=True,
                            )
                            ex = expp.tile([128, 1024], F32R)
                            nc.scalar.activation(ex[:], sc[:], EXP, scale=0.125)
                            if kb >= 4 * qt:
                                # diagonal block: keep exp where q >= k, else 0
                                base = 512 * qt - 128 * kb
                                for half in range(2):
                                    nc.gpsimd.affine_select(
                                        ex[:, half * 512:(half + 1) * 512],
                                        ex[:, half * 512:(half + 1) * 512],
                                        pattern=[[1, 512]],
                                        compare_op=GE,
                                        fill=0.0,
                                        base=base,
                                        channel_multiplier=-1,
                                    )
                            nc.tensor.matmul(
                                avA[:], v_sb[:, kb, 2 * hp, :], ex[:, 0:512],
                                start=(kb == 0), stop=(kb == kmax - 1),
                            )
                            nc.tensor.matmul(
                                avB[:], v_sb[:, kb, 2 * hp + 1, :], ex[:, 512:1024],
                                start=(kb == 0), stop=(kb == kmax - 1),
                            )
                        for j, av in ((0, avA), (1, avB)):
                            rc = rcp.tile([1, 512], F32)
                            nc.vector.reciprocal(rc[:], av[64:65, :])
                            rcb = rcp.tile([64, 512], F32, tag="rcb")
                            nc.gpsimd.partition_broadcast(rcb[:], rc[:])
                            nc.vector.tensor_tensor(
                                ao[j * 64:(j + 1) * 64, hp, :], av[0:64, :], rcb[:], MUL
                            )
                    for m in range(8):
                        po = scp.tile([128, 512], F32, tag="sc")
                        for t in range(4):
                            nc.tensor.matmul(
                                po[:], wo_sb[:, t, m * 128:(m + 1) * 128], ao[:, t, :],
                                start=(t == 0), stop=(t == 3),
                            )
                        oe = oevp.tile([128, 512], F32)
                        nc.vector.tensor_copy(oe[:], po[:])
                        nc.sync.dma_start(
                            outT[m * 128:(m + 1) * 128, qt * 512:(qt + 1) * 512], oe[:]
                        )
    nc.compile()
    return nc


def _get_nc():
    global _NC
    if _NC is None:
        _NC = _build()
    return _NC


def kernel(x, Wa_w, Wa_b, Wo_w, Wo_b):
    x = np.asarray(x, dtype=np.float32)
    Wa_w = np.asarray(Wa_w, dtype=np.float32)
    Wa_b = np.asarray(Wa_b, dtype=np.float32)
    Wo_w = np.asarray(Wo_w, dtype=np.float32)
    Wo_b = np.asarray(Wo_b, dtype=np.float32)

    nc = _get_nc()
    in_maps = []
    for b in range(B):
        xTb = np.ascontiguousarray(x[b].T)
        for hp in range(2):
            sl = slice(hp * 512, (hp + 1) * 512)
            wqk_h = np.ascontiguousarray(
                np.concatenate([Wa_w[0:E][sl], Wa_w[E:2 * E][sl]], axis=0).T
            )
            wv_h = np.ascontiguousarray(Wa_w[2 * E:3 * E][sl].T)
            wo_h = np.ascontiguousarray(Wo_w[:, sl].T)
            bqk_h = np.ascontiguousarray(
                np.concatenate([Wa_b[0:E][sl], Wa_b[E:2 * E][sl]]).reshape(8, 128).T
            )
            bv_h = np.ascontiguousarray(Wa_b[2 * E:3 * E][sl].reshape(1, 512))
            in_maps.append({
                "xT": xTb, "wqk": wqk_h, "wv": wv_h, "wo": wo_h,
                "bqk": bqk_h, "bv": bv_h,
            })

    global _last_in_maps
    _last_in_maps = in_maps
    res = run_bass_kernel_spmd(nc, in_maps, core_ids=list(range(8)))
    out = np.empty((B, S, E), dtype=np.float32)
    for b in range(B):
        acc = res.results[2 * b]["outT"] + res.results[2 * b + 1]["outT"]
        out[b] = acc.T + Wo_b[None, :]
    return out


# revision 24
# speedup vs baseline: 1.1703x; 1.1703x over previous
"""Causal multi-head attention TRN2 kernel.

Problem: B=4, S=2048, E=1024, H=16 heads (D=64), fused QKV projection
(torch-Linear convention), causal softmax attention, output projection.

Sharding (8 NeuronCores): data-parallel over B (4) x tensor-parallel over
heads (2 groups of 8 heads, Megatron-style column/row split of Wa/Wo).
Each core computes a full [S, E] partial of the output projection for its
(batch, head-group); the host sums the two head-group partials per batch
and adds the output bias.

On-core design (all matmuls fp32r = full-rate PE, ~1e-4 rel):
 - QKV projection computed feature-major for Q,K (qk^T = Wqk @ x^T) so the
   attention contraction dims land on SBUF partitions with no transposes,
   and sequence-major for V (v = x @ Wv^T) so V is ready as the AV lhsT.
 - scores^T[k,q] per head-pair: two Kc=64 matmuls packed on PE row groups
   (heads 2t / 2t+1 live on partitions 0:64 / 64:128 of one SBUF tile).
 - softmax without max-subtraction (scores are O(1) bounded for this
   distribution): exp on ACT (PSUM->SBUF eviction for free), causal mask
   applied by GPSIMD affine_select on the diagonal blocks only, softmax
   denominator for free as a 65th ones-column in the AV lhsT.
 - AV accumulates out_h^T[d,q] over key blocks in PSUM; normalization via
   DVE reciprocal + one Kc=1 broadcast matmul + DVE multiply.
 - Output projection consumes the head-major out^T tiles directly.
"""

import numpy as np

import concourse.bass as bass
import concourse.mybir as mybir
import concourse.tile as tile
from concourse import bacc
from concourse.bass_utils import run_bass_kernel_spmd

B, S, E, H = 4, 2048, 1024, 16
D = 64
HPC = 8  # heads per core
F32 = mybir.dt.float32
F32R = mybir.dt.float32r
EXP = mybir.ActivationFunctionType.Exp
ADD = mybir.AluOpType.add
MUL = mybir.AluOpType.mult
GE = mybir.AluOpType.is_ge

_NC = None


def _build():
    nc = bacc.Bacc()
    xT = nc.declare_dram_parameter("xT", [4, 8, 128, 512], F32R, isOutput=False)
    wqk = nc.declare_dram_parameter("wqk", [E, E], F32R, isOutput=False)
    wv = nc.declare_dram_parameter("wv", [E, 512], F32R, isOutput=False)
    wo = nc.declare_dram_parameter("wo", [512, E], F32R, isOutput=False)
    bqk = nc.declare_dram_parameter("bqk", [128, 8], F32, isOutput=False)
    bv = nc.declare_dram_parameter("bv", [1, 512], F32R, isOutput=False)
    outT = nc.declare_dram_parameter("outT", [E, S], F32, isOutput=True)

    with tile.TileContext(nc) as tc:
        with tc.tile_pool(name="persist", bufs=1) as pers:
            # qk_sb[:, t<4, s]  : partitions 0:64 = Q^T of head 2t, 64:128 head 2t+1
            # qk_sb[:, 4+t, s]  : same for K^T
            qk_sb = pers.tile([128, 8, S], F32R)
            # v_sb[p, kb, h, d] : V[kb*128+p, h*64+d], col 64 = 1.0 (denominator)
            v_sb = pers.tile([128, 16, HPC, D + 1], F32R)
            bqk_sb = pers.tile([128, 8], F32)
            bvb_sb = pers.tile([128, 512], F32)
            ones_r = pers.tile([1, 128], F32R)
            onescol = pers.tile([128, 8], F32)
            ones32 = pers.tile([1, 128], F32)

            nc.sync.dma_start(bqk_sb[:], bqk[:])
            nc.vector.memset(ones32[:], 1.0)
            nc.vector.tensor_copy(ones_r[:], ones32[:])
            nc.vector.memset(onescol[:], 1.0)

            expp = tc.alloc_tile_pool(name="expp", bufs=6)
            # ---- Phase 1: fused QKV projection ----
            # scp is allocated ahead of pp so phase-2 scores matmuls never
            # alias phase-1 PSUM banks (no false serialization).
            scp = tc.alloc_tile_pool(name="scp", bufs=2, space="PSUM")
            avp = tc.alloc_tile_pool(name="avp", bufs=2, space="PSUM")
            with tc.tile_pool(name="ph1", bufs=1) as ph1, \
                 tc.tile_pool(name="xch", bufs=2) as xch, \
                 tc.tile_pool(name="pp", bufs=2, space="PSUM") as pp:
                wqk_sb = ph1.tile([128, 8, E], F32R)
                wv_sb = ph1.tile([128, 8, 512], F32R)
                bv_sb = ph1.tile([1, 512], F32R)
                # tiny bv first (first PE op depends on it), then big loads
                nc.sync.dma_start(bv_sb[:], bv[:])
                pb = pp.tile([128, 512], F32, tag="pp")
                nc.tensor.matmul(pb[:], ones_r[:], bv_sb[:], start=True, stop=True)
                nc.vector.tensor_copy(bvb_sb[:], pb[:])
                # warm the ACT exp table set during phase 1
                actwarm = ph1.tile([1, 128], F32)
                nc.scalar.activation(actwarm[:], ones32[:], EXP, scale=0.0)
                # prefetch order: x chunk 0, first-half wqk (f 0:512), x chunk 1,
                # second-half wqk, wv
                def load_xc(si):
                    t = xch.tile([128, 8, 512], F32R, tag="xc")
                    for e in range(8):
                        nc.sync.dma_start(t[:, e, :], xT[si, e])
                    return t

                xc_pre = [load_xc(0)]
                for e in range(8):
                    nc.sync.dma_start(wqk_sb[:, e, 0:512], wqk[e * 128:(e + 1) * 128, 0:512])
                xc_pre.append(load_xc(1))
                for e in range(8):
                    nc.sync.dma_start(wqk_sb[:, e, 512:1024], wqk[e * 128:(e + 1) * 128, 512:1024])
                for e in range(8):
                    nc.sync.dma_start(wv_sb[:, e, :], wv[e * 128:(e + 1) * 128, :])

                for s in range(4):
                    xc = xc_pre[s] if s < 2 else load_xc(s)
                    for f in range(8):
                        pq = pp.tile([128, 512], F32, tag="pp")
                        for e in range(8):
                            nc.tensor.matmul(
                                pq[:],
                                wqk_sb[:, e, f * 128:(f + 1) * 128],
                                xc[:, e, :],
                                start=(e == 0), stop=(e == 7),
                            )
                        nc.vector.tensor_scalar_add(
                            qk_sb[:, f, s * 512:(s + 1) * 512], pq[:], bqk_sb[:, f:f + 1]
                        )
                    for smi in range(4):
                        sm = s * 4 + smi
                        pv = pp.tile([128, 512], F32, tag="pp")
                        for e in range(8):
                            nc.tensor.matmul(
                                pv[:],
                                xc[:, e, smi * 128:(smi + 1) * 128],
                                wv_sb[:, e, :],
                                start=(e == 0), stop=(e == 7),
                            )
                        nc.vector.tensor_tensor(
                            v_sb[:, sm, :, 0:D],
                            pv[:].rearrange("p (h d) -> p h d", h=HPC),
                            bvb_sb[:].rearrange("p (h d) -> p h d", h=HPC),
                            ADD,
                        )
                        nc.vector.tensor_copy(v_sb[:, sm, :, D:D + 1], onescol[:, :, None])

            # ---- Phase 2: attention + output projection ----
            with tc.tile_pool(name="ph2", bufs=1) as ph2, \
                 tc.tile_pool(name="aop", bufs=2) as aop, \
                 tc.tile_pool(name="rcp", bufs=4) as rcp, \
                 tc.tile_pool(name="avsp", bufs=4) as avsp, \
                 tc.tile_pool(name="oevp", bufs=4) as oevp, \
                 tc.tile_pool(name="pop", bufs=2, space="PSUM") as pop:
                wo_sb = ph2.tile([128, 4, E], F32R)
                for t in range(4):
                    nc.sync.dma_start(wo_sb[:, t, :], wo[t * 128:(t + 1) * 128, :])

                def emit_proj(ao, qt, ms):
                    for m in ms:
                        po = pop.tile([128, 512], F32, tag="po")
                        for t in range(4):
                            nc.tensor.matmul(
                                po[:], wo_sb[:, t, m * 128:(m + 1) * 128], ao[:, t, :],
                                start=(t == 0), stop=(t == 3),
                            )
                        oe = oevp.tile([128, 512], F32)
                        nc.vector.tensor_copy(oe[:], po[:])
                        nc.sync.dma_start(
                            outT[m * 128:(m + 1) * 128, qt * 512:(qt + 1) * 512], oe[:]
                        )

                prev = None
                for qt in range(4):
                    ao = aop.tile([128, 4, 512], F32R)
                    for hp in range(4):
                        kmax = 4 * qt + 4

                        def col0(kb, qt=qt):
                            # first allowed q-column (within the 512 block) for key block kb
                            r = kb - 4 * qt
                            return 128 * r if r > 0 else 0

                        def emit_scores(kb, qt=qt, hp=hp):
                            c0 = col0(kb)
                            sc = scp.tile([128, 1024], F32, tag="sc")
                            for half in range(2):
                                nc.tensor.matmul(
                                    sc[:, half * 512 + c0:half * 512 + 512],
                                    qk_sb[half * 64:half * 64 + 64, 4 + hp,
                                          kb * 128:(kb + 1) * 128],
                                    qk_sb[half * 64:half * 64 + 64, hp,
                                          qt * 512 + c0:(qt + 1) * 512],
                                    start=True, stop=True,
                                )
                            return sc

                        avA = avp.tile([65, 512], F32, tag="av")
                        avB = avp.tile([65, 512], F32, tag="av")
                        sc_cur = emit_scores(0)
                        for kb in range(kmax):
                            c0 = col0(kb)
                            sc = sc_cur
                            if kb + 1 < kmax:
                                sc_cur = emit_scores(kb + 1)
                            ex = expp.tile([128, 1024], F32R)
                            # exp only the allowed columns of both head-halves
                            nc.scalar.activation(
                                ex[:].rearrange("p (h q) -> p h q", h=2)[:, :, c0:512],
                                sc[:].rearrange("p (h q) -> p h q", h=2)[:, :, c0:512],
                                EXP, scale=0.125,
                            )
                            if kb >= 4 * qt:
                                # mask the 128-wide strip straddling the diagonal
                                base = 512 * qt - 128 * kb
                                for half in range(2):
                                    nc.gpsimd.affine_select(
                                        ex[:, half * 512 + c0:half * 512 + c0 + 128],
                                        ex[:, half * 512 + c0:half * 512 + c0 + 128],
                                        pattern=[[1, 128]],
                                        compare_op=GE,
                                        fill=0.0,
                                        base=base + c0,
                                        channel_multiplier=-1,
                                    )
                            nc.tensor.matmul(
                                avA[:, c0:512], v_sb[:, kb, 2 * hp, :], ex[:, c0:512],
                                start=(kb == 0), stop=(kb == kmax - 1),
                                skip_group_check=True,
                            )
                            nc.tensor.matmul(
                                avB[:, c0:512], v_sb[:, kb, 2 * hp + 1, :],
                                ex[:, 512 + c0:1024],
                                start=(kb == 0), stop=(kb == kmax - 1),
                                skip_group_check=True,
                            )
                        for j, av in ((0, avA), (1, avB)):
                            # evict AV psum promptly to release the bank
                            avs = avsp.tile([65, 512], F32)
                            nc.vector.tensor_copy(avs[:], av[:])
                            dn = rcp.tile([1, 512], F32, tag="dn")
                            nc.vector.tensor_copy(dn[:], av[64:65, :])
                            rc = rcp.tile([1, 512], F32)
                            nc.vector.reciprocal_approx_fast(rc[:], dn[:])
                            rcb = rcp.tile([64, 512], F32, tag="rcb")
                            nc.gpsimd.partition_broadcast(rcb[:], rc[:])
                            nc.vector.tensor_tensor(
                                ao[j * 64:(j + 1) * 64, hp, :], avs[0:64, :], rcb[:], MUL
                            )
                        if hp in (0, 1) and prev is not None:
                            emit_proj(*prev, range(4 * hp, 4 * hp + 4))
                            if hp == 1:
                                prev = None
                    prev = (ao, qt)
                emit_proj(*prev, range(4))
                emit_proj(*prev, range(4, 8))
            avp.release()
            scp.release()
            expp.release()
    nc.compile()
    return nc


def _get_nc():
    global _NC
    if _NC is None:
        _NC = _build()
    return _NC


def kernel(x, Wa_w, Wa_b, Wo_w, Wo_b):
    x = np.asarray(x, dtype=np.float32)
    Wa_w = np.asarray(Wa_w, dtype=np.float32)
    Wa_b = np.asarray(Wa_b, dtype=np.float32)
    Wo_w = np.asarray(Wo_w, dtype=np.float32)
    Wo_b = np.asarray(Wo_b, dtype=np.float32)

    nc = _get_nc()
    in_maps = []
    for b in range(B):
        xTb = np.ascontiguousarray(x[b].reshape(4, 512, 8, 128).transpose(0, 2, 3, 1))
        for hp in range(2):
            sl = slice(hp * 512, (hp + 1) * 512)
            wqk_h = np.ascontiguousarray(
                np.concatenate([Wa_w[0:E][sl], Wa_w[E:2 * E][sl]], axis=0).T
            )
            wv_h = np.ascontiguousarray(Wa_w[2 * E:3 * E][sl].T)
            wo_h = np.ascontiguousarray(Wo_w[:, sl].T)
            bqk_h = np.ascontiguousarray(
                np.concatenate([Wa_b[0:E][sl], Wa_b[E:2 * E][sl]]).reshape(8, 128).T
            )
            bv_h = np.ascontiguousarray(Wa_b[2 * E:3 * E][sl].reshape(1, 512))
            in_maps.append({
                "xT": xTb, "wqk": wqk_h, "wv": wv_h, "wo": wo_h,
                "bqk": bqk_h, "bv": bv_h,
            })

    global _last_in_maps
    _last_in_maps = in_maps
    res = run_bass_kernel_spmd(nc, in_maps, core_ids=list(range(8)))
    out = np.empty((B, S, E), dtype=np.float32)
    for b in range(B):
        acc = res.results[2 * b]["outT"] + res.results[2 * b + 1]["outT"]
        out[b] = acc.T + Wo_b[None, :]
    return out
